# revision 1
# baseline (speedup 1.0000x reference)
"""Trainium2 Bass kernel for nn_AverageAttention (B=8, L=2048, D=1024).

Math (per batch b):
    avg[t]  = cumsum(x, axis=t)[t] / (t+1)
    g       = concat([x, avg], -1) @ W_gate.T + b_gate        # (L, 2*D)
    out     = sigmoid(g[:, :D]) * x + sigmoid(g[:, D:]) * avg

Strategy: batch-parallel over 8 NeuronCores (one sequence per core), W_gate
replicated. Everything on-chip runs in transposed (feature-on-partition,
token-on-free) layout so the cumulative sum is a single DVE
tensor_tensor_scan per 128-feature chunk. The gating matmul runs in bf16
(fp32 PSUM accumulation); sigmoid + bias is fused into the PSUM evacuation
on the scalar engine. Host pre/post work is limited to layout transposes,
bf16 weight cast, and constant generation.
"""

from contextlib import ExitStack

import ml_dtypes
import numpy as np

import concourse.bass as bass
import concourse.bass_utils as bass_utils
import concourse.mybir as mybir
import concourse.tile as tile
from concourse import bacc
from concourse._compat import with_exitstack
from concourse.bass import ts

B, L, D = 8, 2048, 1024
NJ = D // 128        # 8 feature chunks of x / avg
NK = 2 * D // 128    # 16 contraction chunks of cat = [x, avg]
NOB = 2 * D // 128   # 16 output-feature blocks of g
import os as _os_mod

TCW = int(_os_mod.environ.get("KTCW", "512"))  # matmul moving free-dim
NTC = L // TCW       # token chunks per 2048

FP32 = mybir.dt.float32
BF16 = mybir.dt.bfloat16

# Contraction chunks ordered as phase 1 produces them (x chunk j, then avg
# chunk NJ+j) so the PE can start before phase 1 finishes. W tiles are laid
# out on the host in this order.
KC_ORDER = []
for _j in range(NJ):
    KC_ORDER.extend([_j, NJ + _j])


@with_exitstack
def _tile_body(
    ctx: ExitStack,
    tc: tile.TileContext,
    n_pairs: int = NJ,
    reps: int = 1,
    two_pass: bool = False,
    ph1: int = 2048,
    gp_mul: bool = False,
    w_stat: bool = True,
):
    nc = tc.nc

    xT = nc.dram_tensor("xT", (NJ, 128, L), FP32, kind="ExternalInput").ap()
    wob = nc.dram_tensor("wob", (NOB, 128, NK, 128), BF16, kind="ExternalInput").ap()
    invd = nc.dram_tensor("invd", (128, L), FP32, kind="ExternalInput").ap()
    biash = nc.dram_tensor("biash", (128, NOB), FP32, kind="ExternalInput").ap()
    avgT = nc.dram_tensor("avgT", (NJ, 128, L), FP32, kind="ExternalOutput").ap()
    gatT = nc.dram_tensor("gatT", (NJ, 128, L), FP32, kind="ExternalOutput").ap()

    cat_pool = ctx.enter_context(tc.tile_pool(name="cat", bufs=NK))
    const_pool = ctx.enter_context(tc.tile_pool(name="const", bufs=1))
    x_pool = ctx.enter_context(tc.tile_pool(name="x", bufs=2))
    cum_pool = ctx.enter_context(tc.tile_pool(name="cum", bufs=2))
    avg_pool = ctx.enter_context(tc.tile_pool(name="avg", bufs=2))
    w_pool = ctx.enter_context(tc.tile_pool(name="w", bufs=3))
    sig_pool = ctx.enter_context(tc.tile_pool(name="sig", bufs=3))
    gat_pool = ctx.enter_context(tc.tile_pool(name="gat", bufs=2))
    psum_pool = ctx.enter_context(
        tc.tile_pool(name="psum", bufs=max(1, 8 * 512 // TCW), space="PSUM")
    )
    if two_pass:
        gx_pool = ctx.enter_context(tc.tile_pool(name="gx", bufs=3 * 2 * NTC))
        gs_pool = ctx.enter_context(tc.tile_pool(name="gs", bufs=3))

    invd_sb = const_pool.tile([128, L], FP32, tag="invd")
    bias_sb = const_pool.tile([128, NOB], FP32, tag="bias")

    # cat[kc] for kc in [0, NJ) is bf16 x; [NJ, NK) is bf16 avg.
    cats = [
        cat_pool.tile([128, L], BF16, tag="cat", name=f"cat{k}") for k in range(NK)
    ]

    def load_pair_w(j, chunks=1):
        # W tiles are stored in KC_ORDER on the host. The first pair loads in
        # chunks so the first Ldweights only waits for a quarter of the tile.
        wt_i = w_pool.tile([128, NK, 128], BF16, name="wt_i", tag="wt_i")
        wt_f = w_pool.tile([128, NK, 128], BF16, name="wt_f", tag="wt_f")
        step = NK // chunks
        for c in range(chunks):
            cs = slice(c * step, (c + 1) * step)
            nc.sync.dma_start(wt_i[:, cs, :], wob[j][:, cs, :])
            nc.sync.dma_start(wt_f[:, cs, :], wob[NJ + j][:, cs, :])
        return wt_i, wt_f

    PH1 = ph1  # phase-1 chunk width (DMA efficiency vs pipeline latency)
    NQ1 = L // PH1

    for _rep in range(reps):
        # Head ordering on the sync HWDGE ring (FIFO per ring): first x
        # chunk, then the first pair's W in chunks, so the first matmul
        # unblocks as early as possible. Constants ride the scalar-engine
        # HWDGE ring so they never queue ahead of inputs.
        xt0 = x_pool.tile([128, L], FP32, name="xt", tag="xt")
        nc.sync.dma_start(xt0[:, ts(0, PH1)], xT[0][:, ts(0, PH1)])
        if _rep == 0:
            nc.scalar.dma_start(bias_sb[:], biash[:])
            nc.scalar.dma_start(invd_sb[:], invd[:])
        w_tiles = {j: load_pair_w(j, chunks=4 if j == 0 else 1)
                   for j in range(min(2, n_pairs))}

        # Phase 1: load xT (two DMA chunks), cast the x half of cat per
        # chunk, one full-width cumsum scan + scale per feature block (DVE
        # instruction count kept minimal — per-op drain overhead on the DVE
        # is large on HW), store avg, cast the avg half of cat.
        for j in range(NJ):
            xt = xt0 if j == 0 else x_pool.tile([128, L], FP32, name="xt", tag="xt")
            ct = cum_pool.tile([128, L], FP32)
            at = avg_pool.tile([128, L], FP32)
            for q in range(NQ1):
                s = ts(q, PH1)
                if not (j == 0 and q == 0):
                    nc.sync.dma_start(xt[:, s], xT[j][:, s])
                nc.gpsimd.tensor_copy(cats[j][:, s], xt[:, s])
            nc.vector.tensor_tensor_scan(
                ct[:],
                xt[:],
                xt[:],
                0.0,
                mybir.AluOpType.add,
                mybir.AluOpType.bypass,
            )
            nc.vector.tensor_mul(at[:], ct[:], invd_sb[:])
            nc.gpsimd.tensor_copy(cats[NJ + j][:], at[:])
            nc.sync.dma_start(avgT[j], at[:])

        # Phase 2: gating matmul g^T = W @ cat^T per 128-row output block;
        # sigmoid(g + bias) fused into PSUM evacuation; elementwise gate
        # combine per (input_gate, forget_gate) pair per token chunk.
        #
        # two_pass: the x half of every accumulation runs as an early pass A
        # (no dependence on the cumsum chain), evacuated to bf16 staging; the
        # avg half accumulates later in pass B and is summed with the staging
        # on the DVE before the sigmoid. Pass A for pair j+2 is emitted ahead
        # of pass B for pair j, giving the PE two pairs of phase-1-independent
        # work to hide the cumsum pipeline.
        XI = [i for i, kc in enumerate(KC_ORDER) if kc < NJ]
        AI = [i for i, kc in enumerate(KC_ORDER) if kc >= NJ]

        def pass_a(wts):
            wt_i, wt_f = wts
            gxs = []
            for tcx in range(NTC):
                s = ts(tcx, TCW)
                for wt in (wt_i, wt_f):
                    ps = psum_pool.tile([128, TCW], FP32, name="ps", tag="ps")
                    for n, i in enumerate(XI):
                        nc.tensor.matmul(
                            ps[:],
                            wt[:, i, :],
                            cats[KC_ORDER[i]][:, s],
                            start=(n == 0),
                            stop=(n == len(XI) - 1),
                        )
                    gx = gx_pool.tile([128, TCW], BF16, name="gx", tag="gx")
                    nc.vector.tensor_copy(gx[:], ps[:])
                    gxs.append(gx)
            return gxs

        for j in range(n_pairs, NJ):
            gt = gat_pool.tile([128, L], FP32, name="gt_stub")
            nc.scalar.copy(gt[:], cats[j][:])
            nc.sync.dma_start(gatT[j], gt[:])

        gx_map = {}
        if two_pass:
            for jj in range(min(2, n_pairs)):
                gx_map[jj] = pass_a(w_tiles[jj])
        for j in range(n_pairs):
            if j + 2 < n_pairs:
                w_tiles[j + 2] = load_pair_w(j + 2)
                if two_pass:
                    gx_map[j + 2] = pass_a(w_tiles[j + 2])
            wt_i, wt_f = w_tiles.pop(j)
            gxs = gx_map.pop(j, None)
            gt = gat_pool.tile([128, L], FP32, name="gt", tag="gt")
            st_i = sig_pool.tile([128, L], FP32, name="st", tag="st")
            st_f = sig_pool.tile([128, L], FP32, name="st", tag="st")
            if w_stat and not two_pass:
                # Weight-stationary order: the 4 token chunks run as 4
                # interleaved PSUM groups so consecutive matmuls share the
                # same stationary weights (amortizes the weight-load path).
                for half, wt, st in ((0, wt_i, st_i), (1, wt_f, st_f)):
                    ob = j + NJ * half
                    pss = [
                        psum_pool.tile([128, TCW], FP32, name="ps", tag="ps")
                        for _ in range(NTC)
                    ]
                    for i, kc in enumerate(KC_ORDER):
                        for tcx in range(NTC):
                            nc.tensor.matmul(
                                pss[tcx][:],
                                wt[:, i, :],
                                cats[kc][:, ts(tcx, TCW)],
                                start=(i == 0),
                                stop=(i == NK - 1),
                            )
                    for tcx in range(NTC):
                        nc.scalar.activation(
                            st[:, ts(tcx, TCW)],
                            pss[tcx][:],
                            mybir.ActivationFunctionType.Sigmoid,
                            bias=bias_sb[:, ob : ob + 1],
                        )
            else:
              for tcx in range(NTC):
                s = ts(tcx, TCW)
                for half, wt, st in ((0, wt_i, st_i), (1, wt_f, st_f)):
                    ob = j + NJ * half
                    ps = psum_pool.tile([128, TCW], FP32, name="ps", tag="ps")
                    if two_pass:
                        for n, i in enumerate(AI):
                            nc.tensor.matmul(
                                ps[:],
                                wt[:, i, :],
                                cats[KC_ORDER[i]][:, s],
                                start=(n == 0),
                                stop=(n == len(AI) - 1),
                            )
                        src = gs_pool.tile([128, TCW], FP32, name="gs", tag="gs")
                        nc.vector.tensor_add(src[:], gxs[tcx * 2 + half][:], ps[:])
                    else:
                        for i, kc in enumerate(KC_ORDER):
                            nc.tensor.matmul(
                                ps[:],
                                wt[:, i, :],
                                cats[kc][:, s],
                                start=(i == 0),
                                stop=(i == NK - 1),
                            )
                        src = ps
                    nc.scalar.activation(
                        st[:, s],
                        src[:],
                        mybir.ActivationFunctionType.Sigmoid,
                        bias=bias_sb[:, ob : ob + 1],
                    )
            # Full-width gate combine (3 ops per pair); optionally move the
            # forget-gate product to the otherwise-idle gpsimd engine. The
            # last pair combines and stores per chunk instead, shortening the
            # serial tail after the final matmul.
            if j == n_pairs - 1:
                for tcx in range(NTC):
                    s = ts(tcx, TCW)
                    nc.vector.tensor_mul(gt[:, s], st_i[:, s], cats[j][:, s])
                    nc.vector.tensor_mul(
                        st_f[:, s], st_f[:, s], cats[NJ + j][:, s]
                    )
                    nc.vector.tensor_add(gt[:, s], gt[:, s], st_f[:, s])
                    nc.sync.dma_start(gatT[j][:, s], gt[:, s])
            else:
                nc.vector.tensor_mul(gt[:], st_i[:], cats[j][:])
                if gp_mul:
                    nc.gpsimd.tensor_mul(st_f[:], st_f[:], cats[NJ + j][:])
                else:
                    nc.vector.tensor_mul(st_f[:], st_f[:], cats[NJ + j][:])
                nc.vector.tensor_add(gt[:], gt[:], st_f[:])
                nc.sync.dma_start(gatT[j], gt[:])


_CACHE: dict = {}


def build_nc(
    n_pairs: int | None = None,
    reps: int | None = None,
    two_pass: bool | None = None,
    ph1: int | None = None,
    gp_mul: bool | None = None,
    w_stat: bool | None = None,
):
    import os as _os

    if n_pairs is None:
        n_pairs = int(_os.environ.get("KN_PAIRS", str(NJ)))
    if reps is None:
        reps = int(_os.environ.get("KREPS", "1"))
    if two_pass is None:
        two_pass = _os.environ.get("KTP", "0") == "1"
    if ph1 is None:
        ph1 = int(_os.environ.get("KPH1", "2048"))
    if gp_mul is None:
        gp_mul = _os.environ.get("KGPMUL", "0") == "1"
    if w_stat is None:
        w_stat = _os.environ.get("KWSTAT", "1") == "1"
    key = ("nc", n_pairs, reps, two_pass, TCW, ph1, gp_mul, w_stat)
    if key not in _CACHE:
        nc = bacc.Bacc(
            "TRN2",
            target_bir_lowering=False,
            debug=False,
            enable_asserts=True,
            num_devices=B,
        )
        with tile.TileContext(nc) as t:
            _tile_body(
                t,
                n_pairs=n_pairs,
                reps=reps,
                two_pass=two_pass,
                ph1=ph1,
                gp_mul=gp_mul,
                w_stat=w_stat,
            )
        nc.compile()
        _CACHE[key] = nc
    return _CACHE[key]


def prep_shared(W_gate: np.ndarray, b_gate: np.ndarray):
    # wob[ob, p, i, o] = W_gate[128*ob + o, 128*KC_ORDER[i] + p]
    wob = np.ascontiguousarray(
        W_gate.astype(np.float32)
        .T.reshape(NK, 128, NOB, 128)
        .transpose(2, 1, 0, 3)[:, :, KC_ORDER, :]
    ).astype(ml_dtypes.bfloat16)
    invd = np.ascontiguousarray(
        np.broadcast_to(
            1.0 / np.arange(1, L + 1, dtype=np.float32)[None, :], (128, L)
        )
    )
    biash = np.ascontiguousarray(
        b_gate.astype(np.float32).reshape(NOB, 128).T
    )
    return wob, invd, biash


def kernel(inputs: np.ndarray, W_gate: np.ndarray, b_gate: np.ndarray, **run_kwargs):
    inputs = np.asarray(inputs, dtype=np.float32)
    W_gate = np.asarray(W_gate, dtype=np.float32)
    b_gate = np.asarray(b_gate, dtype=np.float32)
    assert inputs.shape == (B, L, D)

    wob, invd, biash = prep_shared(W_gate, b_gate)
    in_maps = []
    for c in range(B):
        xT_c = np.ascontiguousarray(inputs[c].T).reshape(NJ, 128, L)
        in_maps.append({"xT": xT_c, "wob": wob, "invd": invd, "biash": biash})

    nc = build_nc()
    res = bass_utils.run_bass_kernel_spmd(
        nc, in_maps, core_ids=list(range(B)), **run_kwargs
    )

    gating = np.empty((B, L, D), dtype=np.float32)
    average = np.empty((B, L, D), dtype=np.float32)
    for c in range(B):
        gating[c] = res.results[c]["gatT"].reshape(D, L).T
        average[c] = res.results[c]["avgT"].reshape(D, L).T
    if run_kwargs:
        _CACHE["last_results"] = res
    return gating, average



# revision 3
# speedup vs baseline: 1.6444x; 1.6444x over previous
"""Trainium2 Bass kernel for nn_AverageAttention (B=8, L=2048, D=1024).

Math (per batch b):
    avg[t]  = cumsum(x, axis=t)[t] / (t+1)
    g       = concat([x, avg], -1) @ W_gate.T + b_gate        # (L, 2*D)
    out     = sigmoid(g[:, :D]) * x + sigmoid(g[:, D:]) * avg

Strategy: batch-parallel over 8 NeuronCores (one sequence per core), W_gate
replicated. On-chip layout is transposed (feature-on-partition,
token-on-free) so the cumulative sum is one DVE tensor_tensor_scan per
128-feature chunk.

Gating matmul is mixed precision: the x half of the contraction runs in
bf16 (8 matmuls per 128-row output block), the avg half in fp8-e4m3 with
DoubleRow (4 matmuls contracting 256 rows each). The avg contribution to g
carries ~6% of its variance, so fp8 quantization there is nearly free
(emulated gating rel_l2 1.1e-3, and 5.6e-3 even if the hardware flushes
fp8 subnormals to zero), while DoubleRow halves that half's PE time.
Both halves accumulate into one fp32 PSUM group; sigmoid + bias is fused
into the PSUM evacuation on the scalar engine. Outputs are stored bf16
(adds ~1e-3 rounding, halves output DMA); host converts back to fp32.
"""

from contextlib import ExitStack

import ml_dtypes
import numpy as np

import concourse.bass as bass
import concourse.bass_utils as bass_utils
import concourse.mybir as mybir
import concourse.tile as tile
from concourse import bacc
from concourse._compat import with_exitstack
from concourse.bass import ts

B, L, D = 8, 2048, 1024
NJ = D // 128        # 8 feature chunks of x / avg
NOB = 2 * D // 128   # 16 output-feature blocks of g
NDR = NJ // 2        # 4 DoubleRow steps over the avg half
import os as _os_mod

TCW = int(_os_mod.environ.get("KTCW", "512"))  # matmul moving free-dim
NTC = L // TCW       # token chunks per 2048

FP32 = mybir.dt.float32
BF16 = mybir.dt.bfloat16
FP8 = mybir.dt.float8e4
DR = mybir.MatmulPerfMode.DoubleRow


@with_exitstack
def _tile_body(
    ctx: ExitStack,
    tc: tile.TileContext,
    reps: int = 1,
    ph1: int = 2048,
):
    nc = tc.nc

    xT = nc.dram_tensor("xT", (NJ, 128, L), FP32, kind="ExternalInput").ap()
    wxh = nc.dram_tensor("wxh", (NOB, 128, NJ, 128), BF16, kind="ExternalInput").ap()
    wah = nc.dram_tensor("wah", (NOB, 128, NJ, 128), FP8, kind="ExternalInput").ap()
    invd = nc.dram_tensor("invd", (128, L), FP32, kind="ExternalInput").ap()
    biash = nc.dram_tensor("biash", (128, NOB), FP32, kind="ExternalInput").ap()
    avgT = nc.dram_tensor("avgT", (NJ, 128, L), BF16, kind="ExternalOutput").ap()
    gatT = nc.dram_tensor("gatT", (NJ, 128, L), BF16, kind="ExternalOutput").ap()

    catx_pool = ctx.enter_context(tc.tile_pool(name="catx", bufs=NJ))
    cata_pool = ctx.enter_context(tc.tile_pool(name="cata", bufs=NJ))
    a8_pool = ctx.enter_context(tc.tile_pool(name="a8", bufs=1))
    const_pool = ctx.enter_context(tc.tile_pool(name="const", bufs=1))
    x_pool = ctx.enter_context(tc.tile_pool(name="x", bufs=2))
    cum_pool = ctx.enter_context(tc.tile_pool(name="cum", bufs=2))
    avg_pool = ctx.enter_context(tc.tile_pool(name="avg", bufs=2))
    w_pool = ctx.enter_context(tc.tile_pool(name="w", bufs=3))
    sig_pool = ctx.enter_context(tc.tile_pool(name="sig", bufs=3))
    gat_pool = ctx.enter_context(tc.tile_pool(name="gat", bufs=2))
    psum_pool = ctx.enter_context(
        tc.tile_pool(name="psum", bufs=max(1, 8 * 512 // TCW), space="PSUM")
    )

    invd_sb = const_pool.tile([128, L], FP32, tag="invd")
    bias_sb = const_pool.tile([128, NOB], FP32, tag="bias")

    # Persistent per-rep operand tiles. catx: bf16 x chunks (GEMM + gate
    # combine). cata: bf16 avg chunks (gate combine + avgT store). avg8: the
    # fp8 avg slab, k-chunk dim in the middle so a DoubleRow matmul can slice
    # two adjacent chunks as one 3D AP.
    catxs = [catx_pool.tile([128, L], BF16, tag="catx", name=f"catx{j}") for j in range(NJ)]
    catas = [cata_pool.tile([128, L], BF16, tag="cata", name=f"cata{j}") for j in range(NJ)]
    avg8 = a8_pool.tile([128, NJ, L], FP8, tag="avg8")

    def load_pair_w(j, chunks=1):
        # Weights for output blocks (j, NJ + j): bf16 x half + fp8 avg half.
        # The first pair loads the bf16 half in chunks so the first matmul
        # only waits for a quarter of the tile.
        wx_i = w_pool.tile([128, NJ, 128], BF16, name="wx_i", tag="wx_i")
        wx_f = w_pool.tile([128, NJ, 128], BF16, name="wx_f", tag="wx_f")
        wa_i = w_pool.tile([128, NJ, 128], FP8, name="wa_i", tag="wa_i")
        wa_f = w_pool.tile([128, NJ, 128], FP8, name="wa_f", tag="wa_f")
        step = NJ // chunks
        for c in range(chunks):
            cs = slice(c * step, (c + 1) * step)
            nc.sync.dma_start(wx_i[:, cs, :], wxh[j][:, cs, :])
            nc.sync.dma_start(wx_f[:, cs, :], wxh[NJ + j][:, cs, :])
        nc.sync.dma_start(wa_i[:], wah[j])
        nc.sync.dma_start(wa_f[:], wah[NJ + j])
        return wx_i, wx_f, wa_i, wa_f

    PH1 = ph1  # phase-1 DMA chunk width
    NQ1 = L // PH1

    for _rep in range(reps):
        # Head ordering on the sync HWDGE ring (FIFO per ring): first x
        # chunk, then the first pair's W in chunks, so the first matmul
        # unblocks as early as possible. Constants ride the scalar-engine
        # HWDGE ring so they never queue ahead of inputs.
        xt0 = x_pool.tile([128, L], FP32, name="xt", tag="xt")
        nc.sync.dma_start(xt0[:, ts(0, PH1)], xT[0][:, ts(0, PH1)])
        if _rep == 0:
            nc.scalar.dma_start(bias_sb[:], biash[:])
            nc.scalar.dma_start(invd_sb[:], invd[:])
        w_tiles = {j: load_pair_w(j, chunks=4 if j == 0 else 1) for j in range(2)}

        # Phase 1 per feature chunk j: DMA xT[j], cast x to bf16 (Pool),
        # cumsum scan + 1/(t+1) scale (DVE), cast avg to bf16 (ACT) and fp8
        # (Pool), store bf16 avg. Casts are spread across engines so the DVE
        # scan chain stays the only serial dependency.
        for j in range(NJ):
            xt = xt0 if j == 0 else x_pool.tile([128, L], FP32, name="xt", tag="xt")
            ct = cum_pool.tile([128, L], FP32)
            at = avg_pool.tile([128, L], FP32)
            for q in range(NQ1):
                s = ts(q, PH1)
                if not (j == 0 and q == 0):
                    nc.sync.dma_start(xt[:, s], xT[j][:, s])
                nc.gpsimd.tensor_copy(catxs[j][:, s], xt[:, s])
            nc.vector.tensor_tensor_scan(
                ct[:],
                xt[:],
                xt[:],
                0.0,
                mybir.AluOpType.add,
                mybir.AluOpType.bypass,
            )
            nc.vector.tensor_mul(at[:], ct[:], invd_sb[:])
            nc.scalar.copy(catas[j][:], at[:])
            nc.gpsimd.tensor_copy(avg8[:, j, :], at[:])
            nc.sync.dma_start(avgT[j], catas[j][:])

        # Phase 2 per pair j (output blocks j and NJ+j): weight-stationary
        # over the 4 token chunks; contraction = 8 bf16 x-matmuls then 4
        # DoubleRow fp8 avg-matmuls, all one PSUM accumulation group.
        # sigmoid(g + bias) fused into PSUM evacuation on the scalar engine.
        for j in range(NJ):
            if j + 2 < NJ:
                w_tiles[j + 2] = load_pair_w(j + 2)
            wx_i, wx_f, wa_i, wa_f = w_tiles.pop(j)
            gt = gat_pool.tile([128, L], BF16, name="gt", tag="gt")
            st_i = sig_pool.tile([128, L], FP32, name="st", tag="st")
            st_f = sig_pool.tile([128, L], FP32, name="st", tag="st")
            for half, wx, wa, st in ((0, wx_i, wa_i, st_i), (1, wx_f, wa_f, st_f)):
                ob = j + NJ * half
                pss = [
                    psum_pool.tile([128, TCW], FP32, name="ps", tag="ps")
                    for _ in range(NTC)
                ]
                for i in range(NJ):
                    for tcx in range(NTC):
                        nc.tensor.matmul(
                            pss[tcx][:],
                            wx[:, i, :],
                            catxs[i][:, ts(tcx, TCW)],
                            start=(i == 0),
                            stop=False,
                        )
                for k in range(NDR):
                    kk = slice(2 * k, 2 * k + 2)
                    for tcx in range(NTC):
                        nc.tensor.matmul(
                            pss[tcx][:],
                            wa[:, kk, :],
                            avg8[:, kk, ts(tcx, TCW)],
                            start=False,
                            stop=(k == NDR - 1),
                            perf_mode=DR,
                        )
                for tcx in range(NTC):
                    nc.scalar.activation(
                        st[:, ts(tcx, TCW)],
                        pss[tcx][:],
                        mybir.ActivationFunctionType.Sigmoid,
                        bias=bias_sb[:, ob : ob + 1],
                    )
            # Gate combine on the DVE. The last pair combines and stores per
            # token chunk, shortening the serial tail after the final matmul.
            if j == NJ - 1:
                for tcx in range(NTC):
                    s = ts(tcx, TCW)
                    nc.vector.tensor_mul(gt[:, s], st_i[:, s], catxs[j][:, s])
                    nc.vector.tensor_mul(st_f[:, s], st_f[:, s], catas[j][:, s])
                    nc.vector.tensor_add(gt[:, s], gt[:, s], st_f[:, s])
                    nc.sync.dma_start(gatT[j][:, s], gt[:, s])
            else:
                nc.vector.tensor_mul(gt[:], st_i[:], catxs[j][:])
                nc.vector.tensor_mul(st_f[:], st_f[:], catas[j][:])
                nc.vector.tensor_add(gt[:], gt[:], st_f[:])
                nc.sync.dma_start(gatT[j], gt[:])


_CACHE: dict = {}


def build_nc(
    reps: int | None = None,
    ph1: int | None = None,
):
    import os as _os

    if reps is None:
        reps = int(_os.environ.get("KREPS", "1"))
    if ph1 is None:
        ph1 = int(_os.environ.get("KPH1", "2048"))
    key = ("nc", reps, TCW, ph1)
    if key not in _CACHE:
        nc = bacc.Bacc(
            "TRN2",
            target_bir_lowering=False,
            debug=False,
            enable_asserts=True,
            num_devices=B,
        )
        with tile.TileContext(nc) as t:
            _tile_body(t, reps=reps, ph1=ph1)
        nc.compile()
        _CACHE[key] = nc
    return _CACHE[key]


def prep_shared(W_gate: np.ndarray, b_gate: np.ndarray):
    # wxh[ob, p, i, o] = W_gate[128*ob + o, 128*i + p]          (x half)
    # wah[ob, p, k, o] = W_gate[128*ob + o, D + 128*k + p]      (avg half)
    W = W_gate.astype(np.float32)
    wq = W.T.reshape(2, NJ, 128, NOB, 128).transpose(0, 3, 2, 1, 4)
    wxh = np.ascontiguousarray(wq[0]).astype(ml_dtypes.bfloat16)
    wah = np.ascontiguousarray(wq[1]).astype(ml_dtypes.float8_e4m3)
    invd = np.ascontiguousarray(
        np.broadcast_to(
            1.0 / np.arange(1, L + 1, dtype=np.float32)[None, :], (128, L)
        )
    )
    biash = np.ascontiguousarray(
        b_gate.astype(np.float32).reshape(NOB, 128).T
    )
    return wxh, wah, invd, biash


def make_in_maps(inputs: np.ndarray, W_gate: np.ndarray, b_gate: np.ndarray):
    wxh, wah, invd, biash = prep_shared(W_gate, b_gate)
    in_maps = []
    for c in range(B):
        xT_c = np.ascontiguousarray(inputs[c].T).reshape(NJ, 128, L)
        in_maps.append(
            {"xT": xT_c, "wxh": wxh, "wah": wah, "invd": invd, "biash": biash}
        )
    return in_maps


def kernel(inputs: np.ndarray, W_gate: np.ndarray, b_gate: np.ndarray, **run_kwargs):
    inputs = np.asarray(inputs, dtype=np.float32)
    W_gate = np.asarray(W_gate, dtype=np.float32)
    b_gate = np.asarray(b_gate, dtype=np.float32)
    assert inputs.shape == (B, L, D)

    in_maps = make_in_maps(inputs, W_gate, b_gate)
    nc = build_nc()
    res = bass_utils.run_bass_kernel_spmd(
        nc, in_maps, core_ids=list(range(B)), **run_kwargs
    )

    gating = np.empty((B, L, D), dtype=np.float32)
    average = np.empty((B, L, D), dtype=np.float32)
    for c in range(B):
        gating[c] = res.results[c]["gatT"].astype(np.float32).reshape(D, L).T
        average[c] = res.results[c]["avgT"].astype(np.float32).reshape(D, L).T
    if run_kwargs:
        _CACHE["last_results"] = res
    return gating, average
